# revision 1
# baseline (speedup 1.0000x reference)
"""Trainium2 Bass kernel for nn_MultiHeadAttention_52261162058330.

Reference computes, per (batch, head):
    scores = X @ X.T          # [T, T]
    out    = scores @ X       # [T, D]
with X = x[b, h] of shape [T=2048, D=64], no softmax / no scaling.

Optimizations:
 1. Associativity: out = (X X^T) X = X (X^T X) = X @ G with G = X^T X a
    [64, 64] Gram matrix -> ~32x fewer FLOPs, exact up to summation order.
 2. Split-precision matmuls: X = H + L with H = bf16(X), L = bf16(X - H)
    (covers ~17 mantissa bits).  All matmuls run in bf16 (1 cyc/row on the
    PE + fast weight load vs 4 cyc/row for fp32) accumulating in fp32 PSUM:
      G   = H^T H + H^T L + (H^T L)^T     (drops L^T L ~ 2^-34)
      out = (H + L) @ (Gh + Gl)           (G = Gh + Gl split likewise)
    End-to-end rel error ~ 5e-6 vs the fp32 reference.
 3. Out-stage packing: per row-tile u one [K=128, M=128, N=128] matmul with
    lhsT = [H_u^T ; L_u^T] stacked on K and rhs = [[Gh, Gl], [Gh, Gl]];
    the two N-halves are summed in the epilogue (copy + add).

Sharding: B*H = 32 (batch, head) pairs -> 4 heads per core on 8 cores,
fully independent (no collectives).

Layouts per head (T split as (p u): partition p holds rows 16p..16p+15,
contiguous per partition for DMA):
  hl   [128, 16, 2, 64] bf16 : per tile u: [H_u | L_u]
  xthl [128, 16, 128] bf16   : tile u columns = transpose of [H_u | L_u],
       i.e. rows 0:64 = H_u^T, rows 64:128 = L_u^T
  g2   [128, 2, 64] bf16     : [[Gh, Gl], [Gh, Gl]] (rows duplicated)
"""

import numpy as np

N_CORES = 8
B, H, T, D = 2, 16, 2048, 64
HPC = (B * H) // N_CORES  # heads per core
U = T // 128              # 16 row-tiles per head

_NC = None


def _patch_walrus_flags():
    """Flip --enable-ldw-opt so walrus drops redundant LDWEIGHTS (the
    out-stage issues two matmuls per stationary)."""
    from concourse import bass_utils

    if getattr(bass_utils, "_ldw_patched", False):
        return
    orig = bass_utils.run_command

    def run_command(cmd, *a, **kw):
        if cmd and "walrus_driver" in str(cmd[0]):
            cmd = ["--enable-ldw-opt=true" if c == "--enable-ldw-opt=false" else c
                   for c in cmd]
        return orig(cmd, *a, **kw)

    bass_utils.run_command = run_command
    bass_utils._ldw_patched = True


def _patch_tile_tail():
    """Slim TileContext's exit sequence: drop the second all-engine barrier
    (only needed to fence re-entry, which a kernel tail doesn't have)."""
    from concourse import tile as tile_mod

    if getattr(tile_mod.TileContext, "_tail_patched", False):
        return
    from concourse.tile import ScopedClock

    def _drain_and_barrier(self, tick_clock, wait_clock):
        drain_inst = self.nc.sync.drain()
        wait_clock.add_sem_waits(
            drain_inst.ins, ScopedClock({None: tick_clock.global_clock})
        )
        self.nc.all_engine_barrier()
        popped = self.nc._tile_sem_poison_stack.pop()
        assert popped is self._sem_poison
        self.nc.clear_and_free_semaphores(list(self.sems.allocated().values()))

    tile_mod.TileContext._drain_and_barrier = _drain_and_barrier
    tile_mod.TileContext._tail_patched = True


def _build():
    import concourse.bacc as bacc
    import concourse.mybir as mybir
    from concourse import tile, masks

    _patch_tile_tail()

    nc = bacc.Bacc(
        trn_type="TRN2", target_bir_lowering=False, debug=False,
        num_devices=N_CORES,
    )
    f32 = mybir.dt.float32
    bf16 = mybir.dt.bfloat16
    x_in = nc.dram_tensor("x_shard", [HPC, T, D], f32, kind="ExternalInput").ap()
    y_out = nc.dram_tensor("out_shard", [HPC, T, D], f32, kind="ExternalOutput").ap()
    xv = x_in.rearrange("h (p u) d -> p h u d", p=128)
    yv = y_out.rearrange("h (p u) d -> p h u d", p=128)

    with tile.TileContext(nc) as tc:
        with (
            tc.tile_pool(name="const", bufs=1) as cpool,
            tc.tile_pool(name="iox", bufs=4) as iox,
            tc.tile_pool(name="io", bufs=2) as io,
            tc.tile_pool(name="ios", bufs=4) as ios,
            tc.tile_pool(name="psT", bufs=2, space="PSUM") as psT,
            tc.tile_pool(name="psG", bufs=1, space="PSUM") as psG,
            tc.tile_pool(name="psF", bufs=2, space="PSUM") as psF,
            tc.tile_pool(name="psO", bufs=3, space="PSUM") as psO,
        ):
            identb = cpool.tile([128, 128], bf16)
            masks.make_identity(nc, identb[:])
            identf = cpool.tile([64, 64], f32)
            masks.make_identity(nc, identf[:])

            for h in range(HPC):
                xsb = iox.tile([128, U, D], f32, tag="xsb")
                hl = io.tile([128, U, 2, D], bf16, tag="hl")
                chunks = (0, 8, U) if h == 0 else (0, U)
                for c in range(len(chunks) - 1):
                    sl = slice(chunks[c], chunks[c + 1])
                    nc.sync.dma_start(out=xsb[:, sl], in_=xv[:, h, sl])
                    nc.vector.tensor_copy(hl[:, sl, 0, :], xsb[:, sl])
                    nc.vector.tensor_sub(hl[:, sl, 1, :], xsb[:, sl], hl[:, sl, 0, :])

                # xthl[:, u, :] = [H_u | L_u]^T  (rows 0:64 H^T, 64:128 L^T)
                xthl = io.tile([128, U, 128], bf16, tag="xthl")
                for q in range(U // 4):
                    pst = psT.tile([128, 4, 128], bf16, tag="pst")
                    for i in range(4):
                        u = 4 * q + i
                        nc.tensor.transpose(pst[:, i, :], hl[:, u].rearrange("p a b -> p (a b)"), identb[:])
                    if q % 4 < 3:
                        nc.vector.tensor_copy(xthl[:, 4 * q:4 * q + 4, :], pst[:])
                    else:
                        nc.scalar.copy(xthl[:, 4 * q:4 * q + 4, :], pst[:])

                # G partials: psg[:, 0] = sum H_u^T H_u, psg[:, 1] = sum H_u^T L_u
                psg = psG.tile([64, 2, D], f32, tag="psg")
                for u in range(U):
                    nc.tensor.matmul(
                        psg[:].rearrange("p a b -> p (a b)"),
                        hl[:, u, 0, :],
                        hl[:, u].rearrange("p a b -> p (a b)"),
                        start=(u == 0), stop=(u == U - 1),
                    )
                shl = ios.tile([64, 2, D], f32, tag="shl")
                nc.vector.tensor_copy(shl[:], psg[:])
                # G = HH + HL + HL^T  (HL^T via PE)
                pft = psF.tile([64, D], f32, tag="pf")
                nc.tensor.transpose(pft[:], shl[:, 1, :], identf[:])
                gf = ios.tile([64, D], f32, tag="gf")
                nc.vector.tensor_add(gf[:], shl[:, 0, :], shl[:, 1, :])
                nc.vector.tensor_add(gf[:], gf[:], pft[:])

                # split G; gcat = [Gh | Gl], gcat2 = [Gl | Gh]
                gcat = ios.tile([64, 2, D], bf16, tag="gcat")
                nc.scalar.copy(gcat[:, 0, :], gf[:])
                nc.vector.tensor_sub(gcat[:, 1, :], gf[:], gcat[:, 0, :])
                gcat2 = ios.tile([64, 2, D], bf16, tag="gcat2")
                nc.scalar.copy(gcat2[:, 1, :], gf[:])
                nc.vector.tensor_sub(gcat2[:, 0, :], gf[:], gcat2[:, 1, :])
                # transpose(gcat)  -> rows 64:128 = Gl  (partitions 64:128)
                # transpose(gcat2) -> rows 64:128 = Gh  (partitions 64:128)
                pgt = psF.tile([128, 2, D], bf16, tag="pf")
                nc.tensor.transpose(
                    pgt[:, 0, :], gcat[:].rearrange("p a b -> p (a b)"), identb[0:64, 0:64]
                )
                nc.tensor.transpose(
                    pgt[:, 1, :], gcat2[:].rearrange("p a b -> p (a b)"), identb[0:64, 0:64]
                )
                # g2[:, 0, :] = [Gh; Gh], g2[:, 1, :] = [Gl; Gl] (K-stacked)
                g2 = ios.tile([128, 2, D], bf16, tag="g2")
                nc.vector.tensor_copy(g2[0:64, :, :], gcat[:])
                nc.scalar.copy(g2[64:128, 0, :], pgt[64:128, 1, :])
                nc.scalar.copy(g2[64:128, 1, :], pgt[64:128, 0, :])

                # out tiles: per u one [K=128, M=128, N=128] MM with
                # rhs = [[Gh, Gl], [Gh, Gl]]; halves summed per 4-tile bank
                osb = io.tile([128, U, D], f32, tag="osb")
                for q in range(4):
                    pso = psO.tile([128, 4, 2, D], f32, tag="pso")
                    for i in range(4):
                        u = 4 * q + i
                        nc.tensor.matmul(
                            pso[:, i].rearrange("p a b -> p (a b)"),
                            xthl[:, u, :],
                            g2[:].rearrange("p a b -> p (a b)"),
                            start=True, stop=True,
                        )
                    tmp = ios.tile([128, 4, D], f32, tag="otmp")
                    nc.scalar.copy(tmp[:], pso[:, :, 1, :])
                    nc.vector.tensor_add(
                        osb[:, 4 * q:4 * q + 4, :], pso[:, :, 0, :], tmp[:]
                    )
                    if h == HPC - 1:
                        nc.sync.dma_start(
                            out=yv[:, h, 4 * q:4 * q + 4], in_=osb[:, 4 * q:4 * q + 4]
                        )

                if h != HPC - 1:
                    nc.sync.dma_start(out=yv[:, h], in_=osb[:])

    nc.compile()
    return nc


def _get_nc():
    global _NC
    if _NC is None:
        _NC = _build()
    return _NC


def kernel(x: np.ndarray) -> np.ndarray:
    from concourse.bass_utils import run_bass_kernel_spmd

    assert x.shape == (B, H, T, D), x.shape
    x_flat = np.ascontiguousarray(x.reshape(B * H, T, D), dtype=np.float32)
    in_maps = [
        {"x_shard": np.ascontiguousarray(x_flat[c * HPC:(c + 1) * HPC])}
        for c in range(N_CORES)
    ]
    res = run_bass_kernel_spmd(_get_nc(), in_maps, list(range(N_CORES)))
    out = np.concatenate([res.results[c]["out_shard"] for c in range(N_CORES)], axis=0)
    return out.reshape(B, H, T, D)



# revision 8
# speedup vs baseline: 1.2623x; 1.2623x over previous
"""Trainium2 Bass kernel for nn_MultiHeadAttention_52261162058330.

Reference computes, per (batch, head):
    scores = X @ X.T          # [T, T]
    out    = scores @ X       # [T, D]
with X = x[b, h] of shape [T=2048, D=64], no softmax / no scaling.

Optimizations:
 1. Associativity: out = (X X^T) X = X (X^T X) = X @ G with G = X^T X a
    [64, 64] Gram matrix -> ~32x fewer FLOPs, exact up to summation order.
 2. fp16 everywhere (1 cyc/row on the PE vs 4 for fp32), fp32 PSUM
    accumulation.  X ~ N(0,1), |out| < 1e4 << fp16 max, so no overflow;
    end-to-end rel l2 error ~ 4e-4 vs the fp32 reference (budget 2e-2).
 3. Pair-packed PE transposes: one [128,128] transpose yields X_u^T and
    X_{u+1}^T stacked on partitions (halves transpose cycles).
 4. Output stored fp16 (halves output DMA); host upcasts to fp32.
 5. Copies / dtype converts spread across DVE + Act + Pool engines.

Sharding: B*H = 32 (batch, head) pairs -> 4 heads per core on 8 cores,
fully independent (no collectives).

Layouts per head (T split as (p u): partition p holds rows 16p..16p+15,
contiguous per partition for DMA):
  xh [128, 16, 64] f16 : row tiles X_u
  xt [128, 8, 128] f16 : pair q holds [X_2q^T ; X_2q+1^T] stacked on
       partitions (0:64 even tile, 64:128 odd tile)
  gh [128, 64] f16     : G duplicated on both partition halves
"""

import numpy as np

N_CORES = 8
B, H, T, D = 2, 16, 2048, 64
HPC = (B * H) // N_CORES  # heads per core
U = T // 128              # 16 row-tiles per head
Q = U // 2                # 8 transpose pairs per head

_NC = None


def _patch_tile_tail():
    """Slim TileContext's exit sequence: drop the second all-engine barrier
    (only needed to fence re-entry, which a kernel tail doesn't have)."""
    from concourse import tile as tile_mod

    if getattr(tile_mod.TileContext, "_tail_patched", False):
        return
    from concourse.tile import ScopedClock

    def _drain_and_barrier(self, tick_clock, wait_clock):
        drain_inst = self.nc.sync.drain()
        wait_clock.add_sem_waits(
            drain_inst.ins, ScopedClock({None: tick_clock.global_clock})
        )
        self.nc.all_engine_barrier()
        popped = self.nc._tile_sem_poison_stack.pop()
        assert popped is self._sem_poison
        self.nc.clear_and_free_semaphores(list(self.sems.allocated().values()))

    tile_mod.TileContext._drain_and_barrier = _drain_and_barrier
    tile_mod.TileContext._tail_patched = True


def _build():
    import concourse.bacc as bacc
    import concourse.mybir as mybir
    from concourse import tile, masks

    _patch_tile_tail()

    nc = bacc.Bacc(
        trn_type="TRN2", target_bir_lowering=False, debug=False,
        num_devices=N_CORES,
    )
    f32 = mybir.dt.float32
    f16 = mybir.dt.float16
    x_in = nc.dram_tensor("x_shard", [HPC, T, D], f32, kind="ExternalInput").ap()
    y_out = nc.dram_tensor("out_shard", [HPC, T, D], f16, kind="ExternalOutput").ap()
    xv = x_in.rearrange("h (p u) d -> p h u d", p=128)
    yv = y_out.rearrange("h (p u) d -> p h u d", p=128)

    with tile.TileContext(nc) as tc:
        with (
            tc.tile_pool(name="const", bufs=1) as cpool,
            tc.tile_pool(name="iof", bufs=2) as iof,
            tc.tile_pool(name="ioh", bufs=2) as ioh,
            tc.tile_pool(name="iot", bufs=2) as iot,
            tc.tile_pool(name="iog", bufs=2) as iog,
            tc.tile_pool(name="ioo", bufs=2) as ioo,
            tc.tile_pool(name="psG", bufs=2, space="PSUM") as psG,
            tc.tile_pool(name="psT", bufs=2, space="PSUM") as psT,
            tc.tile_pool(name="psD", bufs=1, space="PSUM") as psD,
            tc.tile_pool(name="psO", bufs=3, space="PSUM") as psO,
        ):
            ident = cpool.tile([128, 128], f16)
            masks.make_identity(nc, ident[:])
            # idup = [I64 | I64]: duplicates G onto both partition halves
            idup = cpool.tile([64, 2, 64], f16)
            masks.make_identity(nc, idup[:, 0, :])
            masks.make_identity(nc, idup[:, 1, :])
            # gstack[h % 2] holds [[G, 0], [0, G]]; off-diagonal zeros are
            # written once here, only the diagonal blocks change per head
            gstack = [cpool.tile([128, 2, 64], f16, name=f"gstack{i}")
                      for i in range(2)]
            for g in gstack:
                nc.gpsimd.memset(g[:], 0.0)

            for h in range(HPC):
                xf = iof.tile([128, U, D], f32, tag="xf")
                xh = ioh.tile([128, U, D], f16, tag="xh")
                # two DMA chunks per head so the convert starts earlier.
                # Pool (gpsimd) owns the fp32->fp16 convert in steady state
                # (it cannot read PSUM, so it gets the SBUF-only work); on
                # the first head all three engines pitch in for latency.
                for c0, c1 in ((0, U // 2), (U // 2, U)):
                    nc.sync.dma_start(out=xf[:, c0:c1], in_=xv[:, h, c0:c1])
                    if h == 0:
                        t0, t1 = c0 + 2, c0 + 5
                        nc.vector.tensor_copy(xh[:, c0:t0], xf[:, c0:t0])
                        nc.scalar.copy(xh[:, t0:t1], xf[:, t0:t1])
                        nc.gpsimd.tensor_copy(xh[:, t1:c1], xf[:, t1:c1])
                    else:
                        nc.gpsimd.tensor_copy(xh[:, c0:c1], xf[:, c0:c1])

                # G = sum_u X_u^T X_u  (fp32 PSUM accumulation)
                psg = psG.tile([64, D], f32, tag="psg")
                for u in range(U):
                    nc.tensor.matmul(
                        psg[:], xh[:, u], xh[:, u],
                        start=(u == 0), stop=(u == U - 1),
                    )

                # pair transposes: pst[:, q] = [X_2q | X_2q+1]^T
                pst = psT.tile([128, Q, 128], f16, tag="pst")
                for q in range(Q):
                    nc.tensor.transpose(
                        pst[:, q],
                        xh[:, 2 * q:2 * q + 2].rearrange("p a b -> p (a b)"),
                        ident[:],
                    )
                xt = iot.tile([128, Q, 128], f16, tag="xt")
                nc.vector.tensor_copy(xt[:, 0:Q // 2], pst[:, 0:Q // 2])
                nc.vector.tensor_copy(xt[:, Q // 2:Q], pst[:, Q // 2:Q])

                # G -> fp16, duplicated onto partitions 64:128 via PE, then
                # scattered into the block-diagonal gstack = [[G, 0], [0, G]]
                gh0 = iog.tile([64, D], f16, tag="gh0")
                nc.vector.tensor_copy(gh0[:], psg[:])
                gd = psD.tile([128, D], f32, tag="gd")
                nc.tensor.matmul(
                    gd[:], idup[:].rearrange("p a b -> p (a b)"), gh0[:],
                    start=True, stop=True,
                )
                gh = iog.tile([128, D], f16, tag="gh")
                nc.vector.tensor_copy(gh[:], gd[:])
                gs = gstack[h % 2]
                nc.vector.tensor_copy(gs[0:64, 0, :], gh[0:64, :])
                nc.vector.tensor_copy(gs[64:128, 1, :], gh[64:128, :])

                # out pairs: [out_2q | out_2q+1] = [X_2q^T ; X_2q+1^T]^T @ gstack
                of = ioo.tile([128, U, D], f16, tag="of")
                for c in range(2):
                    pso = psO.tile([128, Q // 2, 2, D], f32, tag="pso")
                    for j in range(Q // 2):
                        q = (Q // 2) * c + j
                        nc.tensor.matmul(
                            pso[:, j].rearrange("p a b -> p (a b)"),
                            xt[:, q, :],
                            gs[:].rearrange("p a b -> p (a b)"),
                            start=True, stop=True,
                        )
                    sl = slice((U // 2) * c, (U // 2) * (c + 1))
                    nc.scalar.copy(of[:, sl], pso[:].rearrange("p a b c -> p (a b) c"))
                    nc.sync.dma_start(out=yv[:, h, sl], in_=of[:, sl])

    nc.compile()
    return nc


def _get_nc():
    global _NC
    if _NC is None:
        _NC = _build()
    return _NC


def kernel(x: np.ndarray) -> np.ndarray:
    from concourse.bass_utils import run_bass_kernel_spmd

    assert x.shape == (B, H, T, D), x.shape
    x_flat = np.ascontiguousarray(x.reshape(B * H, T, D), dtype=np.float32)
    in_maps = [
        {"x_shard": np.ascontiguousarray(x_flat[c * HPC:(c + 1) * HPC])}
        for c in range(N_CORES)
    ]
    res = run_bass_kernel_spmd(_get_nc(), in_maps, list(range(N_CORES)))
    out = np.concatenate(
        [res.results[c]["out_shard"] for c in range(N_CORES)], axis=0
    )
    return out.reshape(B, H, T, D).astype(np.float32)
